# revision 9
# baseline (speedup 1.0000x reference)
"""Distributed multi-head attention kernel for 8 TRN2 NeuronCores.

Problem: x[4, 2048, 1024] @ w_qkv[1024, 3072] -> qkv -> 16-head attention
         -> out[4, 2048, 1024], fp32.

Sharding (data parallel batch x tensor parallel heads):
  core c handles batch b = c // 2 and heads h0 = (c % 2) * 8 .. h0 + 8.
  Each core receives x_b [2048, 1024] and the w_qkv column slice for its
  8 heads ([1024, 1536] = q|k|v each 512 cols), produces out[b, :, 512-slice].
  All 64 (batch, head) attention problems are independent -> no collectives.

v3: x and w are cast to bf16 on the HOST (numerically identical to the
previous on-chip cast) so that
  - xT is produced by 32 hardware xbar DMA-transposes (dma_start_transpose)
    straight from DRAM -- no PE transposes, no DVE casts/evacuations;
  - wsb is loaded by 12 plain column-block DMAs -- no casts.
This empties the x/w feed off all compute engines; the scalar-engine Exp
stream (256 ACTIVATEs of [128,1024] ~ 1us each) is the pacing floor and
the PE j-step work (dots pair + PV pair + background QKV) is kept below it.
"""

import numpy as np

B, N, DIM = 4, 2048, 1024
HEADS, DIM_HEAD = 16, 64
INNER = HEADS * DIM_HEAD
HPC = 8                 # heads per core
FQ = HPC * DIM_HEAD     # 512 = per-core q/k/v column count
NCORES = 8

P = 128
CT = DIM // P           # 8 c-tiles (contraction)
IT = N // P             # 16 i-tiles
JT = N // P             # 16 j-tiles

_CACHE = {}


def _build():
    import concourse.bass as bass
    import concourse.mybir as mybir
    import concourse.tile as tile
    from concourse import bacc
    from concourse.masks import make_identity

    f32 = mybir.dt.float32
    bf16 = mybir.dt.bfloat16
    Exp = mybir.ActivationFunctionType.Exp
    ds = bass.ds

    nc = bacc.Bacc(None, target_bir_lowering=False)
    x_d = nc.dram_tensor("x", [N, DIM], bf16, kind="ExternalInput")
    w_d = nc.dram_tensor("w", [DIM, 3 * FQ], bf16, kind="ExternalInput")
    o_d = nc.dram_tensor("o", [N, FQ], f32, kind="ExternalOutput")

    # w viewed as [partition, c-tile, f]
    w_v = w_d[:].rearrange("(c p) f -> p c f", p=P)

    with tile.TileContext(nc) as tc, \
         tc.tile_pool(name="persist", bufs=1) as persist, \
         tc.tile_pool(name="xload", bufs=4) as xload, \
         tc.tile_pool(name="qkvp", bufs=1, space="PSUM") as qkvp, \
         tc.tile_pool(name="ptp", bufs=6) as ptp, \
         tc.tile_pool(name="uep", bufs=5) as uep, \
         tc.tile_pool(name="recp", bufs=4) as recp, \
         tc.tile_pool(name="normp", bufs=8) as normp:

        ident = persist.tile([P, P], bf16, tag="ident", name="ident")
        make_identity(nc, ident[:])

        xT = persist.tile([P, CT, N], bf16, tag="xT", name="xT")
        wsb = persist.tile([P, CT, 3 * FQ], bf16, tag="wsb", name="wsb")
        qkT = persist.tile([P, CT, N], bf16, tag="qkT", name="qkT")
        vp = persist.tile([P, JT, HPC * 65 + 63], bf16, tag="vp", name="vp")

        vp_heads = vp[:, :, 0:HPC * 65].rearrange("p j (h c) -> p j h c", c=65)
        nc.vector.memset(vp_heads[:, :, :, 64:65], 1.0)
        nc.vector.memset(vp[:, :, HPC * 65:], 0.0)

        # ---- feeds: pure DMA (hw xbar transpose) for i-chunks 1-3; the
        # first chunk goes through PE transpose-mode so the first q/k
        # projection isn't gated on the slow DMA-transpose issue path.
        def emit_xT_dma(ic):          # one 512-row i-chunk, all c-tiles
            for ct in range(CT):
                nc.sync.dma_start_transpose(
                    xT[:, ct, ds(ic * 512, 512)],
                    x_d[ds(ic * 512, 512), ds(ct * P, P)])

        def emit_x0_tile(it, tpsx):   # chunk-0 path: plain DMA + PE transpose
            xf = xload.tile([P, DIM], bf16, tag="xf")
            nc.sync.dma_start(xf[:], x_d[ds(it * P, P), :])
            for ch in range(2):
                tp4 = tpsx.tile([P, 4, P], bf16, tag="tpsx")
                for k in range(4):
                    nc.tensor.transpose(
                        tp4[:, k], xf[:, ds((ch * 4 + k) * P, P)], ident[:])
                nc.vector.tensor_copy(
                    xT[:, ds(ch * 4, 4), ds(it * P, P)], tp4[:])

        def emit_w_dma(blk):          # one 128-column f block
            nc.sync.dma_start(
                wsb[:, :, ds(blk * P, P)], w_v[:, :, ds(blk * P, P)])

        def emit_qk_group(ft, ic, cts=range(CT)):
            ps = emit_qk_group.ps
            if 0 in cts:
                ps = qkvp.tile([P, 512], f32, tag="qkv", name="qkv_ps")
                emit_qk_group.ps = ps
            for ct in cts:
                nc.tensor.matmul(
                    ps[:],
                    wsb[:, ct, ds(ft * P, P)],
                    xT[:, ct, ds(ic * 512, 512)],
                    start=(ct == 0), stop=(ct == CT - 1),
                )
            if CT - 1 in cts:
                nc.vector.tensor_copy(qkT[:, ft, ds(ic * 512, 512)], ps[:])
        emit_qk_group.ps = None

        def emit_v_group(hp, it):
            ps = qkvp.tile([P, P], f32, tag="qkv")
            for ct in range(CT):
                nc.tensor.matmul(
                    ps[:],
                    xT[:, ct, ds(it * P, P)],
                    wsb[:, ct, ds(2 * FQ + hp * P, P)],
                    start=(ct == 0), stop=(ct == CT - 1),
                )
            nc.vector.tensor_copy(
                vp_heads[:, it, ds(2 * hp, 2), 0:64],
                ps[:].rearrange("p (h c) -> p h c", c=64),
            )

        def emit_epi_evac(ups):
            # free the U' psum slot ASAP; finish is deferred into the next
            # i-chunk's j-loop so the transposes don't delay its first exps
            ue = uep.tile([65, 512], bf16, tag="ue")
            nc.vector.tensor_copy(ue[:], ups[0:65, :])
            return ue

        def emit_epi_finish(ue, ic, h, tpp):
            # [d|Z, 512] -> 4 matmul-transposes into ONE psum tile (no
            # per-block WAR serialization) -> batched recip -> 4 normalizes
            tp = tpp.tile([P, 4, 65], f32, tag="tpsx")
            for b in range(4):
                nc.tensor.matmul(
                    tp[:, b], ue[:, ds(b * P, P)], ident[0:65, 0:65],
                    start=True, stop=True)
            rec = recp.tile([P, 4], f32, tag="rec")
            nc.vector.reciprocal(rec[:], tp[:, :, 64])
            for b in range(4):
                nrm = normp.tile([P, 64], f32, tag="nrm")
                nc.vector.tensor_scalar_mul(nrm[:], tp[:, b, 0:64],
                                            rec[:, ds(b, 1)])
                it = ic * 4 + b
                nc.sync.dma_start(
                    o_d[ds(it * P, P), ds(h * 64, 64)], nrm[:]
                )

        def emit_attention_pair(hp, dotsp, upp, tpp, bg_emit=None):
            # heads hA = 2hp (partitions 0:64), hB = 2hp+1 (64:128) share
            # f-tiles qft/kft; dots for both packed into one [128, 1024]
            # psum (column halves, concurrent row-tiled matmuls).
            hA, hB = 2 * hp, 2 * hp + 1
            qft, kft = hp, 4 + hp
            pending = []
            carry = []
            for ic in range(4):          # i-chunks of 512
                upsA = upp.tile([P, 512], f32, tag="upsA")
                upsB = upp.tile([P, 512], f32, tag="upsB")
                pts = {}

                def emit_pv(jj, upsA=upsA, upsB=upsB, pts=pts):
                    nc.tensor.matmul(
                        upsA[:], vp[:, jj, ds(hA * 65, 128)],
                        pts[jj][:, 0:512],
                        start=(jj == 0), stop=(jj == JT - 1),
                    )
                    nc.tensor.matmul(
                        upsB[:], vp[:, jj, ds(hB * 65, 128)],
                        pts[jj][:, 512:1024],
                        start=(jj == 0), stop=(jj == JT - 1),
                    )
                    del pts[jj]

                def emit_dots_exp(j):
                    dt_ = dotsp.tile([P, 1024], f32, tag="dt")
                    nc.tensor.matmul(
                        dt_[:, 0:512],
                        qkT[0:64, kft, ds(j * P, P)],
                        qkT[0:64, qft, ds(ic * 512, 512)],
                        start=True, stop=True,
                    )
                    nc.tensor.matmul(
                        dt_[:, 512:1024],
                        qkT[64:128, kft, ds(j * P, P)],
                        qkT[64:128, qft, ds(ic * 512, 512)],
                        start=True, stop=True,
                    )
                    pt = ptp.tile([P, 1024], bf16, tag="pt")
                    nc.scalar.activation(pt[:], dt_[:], Exp, scale=0.125)
                    pts[j] = pt

                # 2-step blocks: consecutive dots pairs adjacent on the PE
                # queue, then the two lagged PV pairs. The previous
                # i-chunk's last two PV pairs + U' evacuations are carried
                # into this chunk's first block so the boundary dots (which
                # wait on the exp two steps back for their psum slot) have
                # PE work in front of them and the exp stream stays dense.
                for j2 in range(0, JT, 2):
                    if bg_emit is not None:
                        bg_emit(ic, j2)
                        bg_emit(ic, j2 + 1)
                    emit_dots_exp(j2)
                    emit_dots_exp(j2 + 1)
                    if j2 == 0:
                        for fn in carry:
                            fn()
                        carry = []
                    if j2 == 6 and pending:
                        emit_epi_finish(*pending.pop(0))
                    if j2 == 10 and pending:
                        emit_epi_finish(*pending.pop(0))
                    if j2 >= 2:
                        emit_pv(j2 - 2)
                        emit_pv(j2 - 1)
                carry = [
                    lambda e=emit_pv: e(JT - 2),
                    lambda e=emit_pv: e(JT - 1),
                    lambda u=upsA, i=ic: pending.append(
                        (emit_epi_evac(u), i, hA, tpp)),
                    lambda u=upsB, i=ic: pending.append(
                        (emit_epi_evac(u), i, hB, tpp)),
                ]
            for fn in carry:
                fn()
            while pending:
                emit_epi_finish(*pending.pop(0))

        # ---- emission ----
        with tc.tile_pool(name="tpsx", bufs=1, space="PSUM") as tpsx, \
             tc.tile_pool(name="dotsp", bufs=2, space="PSUM") as dotsp, \
             tc.tile_pool(name="upp", bufs=1, space="PSUM") as upp:
            # startup: i-chunk 0 of x + pair-0 w blocks by plain DMA, PE
            # transposes for chunk 0, xbar DMA-transposes for chunks 1-3,
            # first q/k chunk, first two v tiles.
            for it in range(4):
                emit_x0_tile(it, tpsx)
            emit_w_dma(0)            # pair-0 q cols
            emit_w_dma(4)            # pair-0 k cols
            emit_w_dma(8)            # pair-0 v cols
            for ic in range(1, 4):
                emit_xT_dma(ic)
            emit_qk_group(0, 0)
            emit_qk_group(4, 0)
            for it in range(2):
                emit_v_group(0, it)

            def bg_pair0(ic, j):
                # k-side chunks early (dots j consumes kT chunk j//4),
                # q-side chunks late (chunk b first used at i-chunk b).
                if ic != 0:
                    return
                if j in (1, 3, 5):
                    emit_qk_group(4, (j + 1) // 2)
                if j in (9, 11, 13):
                    emit_qk_group(0, (j - 7) // 2)
                if 2 <= j + 1 < JT:
                    emit_v_group(0, j + 1)

            def bg_next_pair(nhp):
                # pair nhp's w DMAs + QKV groups interleaved into the
                # previous pair's attention (ic 1-3); qk groups split in
                # half-contraction units to bound the PE lump per slot.
                groups = [
                    lambda: emit_w_dma(nhp),
                    lambda: emit_w_dma(4 + nhp),
                    lambda: emit_w_dma(8 + nhp),
                ]
                for icq in range(4):
                    for ft in (nhp, 4 + nhp):
                        groups.append(
                            lambda ft=ft, icq=icq:
                            emit_qk_group(ft, icq, cts=range(0, 4)))
                        groups.append(
                            lambda ft=ft, icq=icq:
                            emit_qk_group(ft, icq, cts=range(4, 8)))
                for itv in range(IT):
                    groups.append(lambda itv=itv: emit_v_group(nhp, itv))
                gi = {"i": 0}

                def bg(ic, j):
                    if ic == 0:
                        return
                    if gi["i"] < len(groups):
                        groups[gi["i"]]()
                        gi["i"] += 1
                bg.flush = lambda: [g() for g in groups[gi["i"]:]]
                return bg

            bg = bg_pair0
            for hp in range(4):
                nxt = bg_next_pair(hp + 1) if hp < 3 else None
                combined = bg
                if nxt is not None:
                    prev = bg

                    def combined(ic, j, prev=prev, nxt=nxt):
                        if prev is not None:
                            prev(ic, j)
                        nxt(ic, j)
                emit_attention_pair(hp, dotsp, upp, tpsx, bg_emit=combined)
                if nxt is not None:
                    nxt.flush()
                bg = None

    nc.finalize()
    return nc


def _get_nc():
    if "nc" not in _CACHE:
        _CACHE["nc"] = _build()
    return _CACHE["nc"]


def make_in_maps(x: np.ndarray, w_qkv: np.ndarray) -> list:
    import ml_dtypes

    bf16 = ml_dtypes.bfloat16
    x = np.asarray(x, dtype=np.float32).astype(bf16)
    w_qkv = np.asarray(w_qkv, dtype=np.float32).astype(bf16)

    in_maps = []
    for c in range(NCORES):
        b, hh = c // 2, c % 2
        qo = hh * FQ
        ws = np.concatenate(
            [w_qkv[:, qo:qo + FQ],
             w_qkv[:, INNER + qo:INNER + qo + FQ],
             w_qkv[:, 2 * INNER + qo:2 * INNER + qo + FQ]], axis=1)
        in_maps.append({
            "x": np.ascontiguousarray(x[b]),
            "w": np.ascontiguousarray(ws),
        })
    return in_maps


def kernel(x: np.ndarray, w_qkv: np.ndarray) -> np.ndarray:
    from concourse.bass_utils import run_bass_kernel_spmd

    in_maps = make_in_maps(x, w_qkv)
    nc = _get_nc()
    res = None
    last_err = None
    for attempt in range(3):
        try:
            res = run_bass_kernel_spmd(nc, in_maps, core_ids=list(range(NCORES)))
            break
        except Exception as e:  # transient axon/NRT device errors
            last_err = e
            import time
            time.sleep(20 * (attempt + 1))
    if res is None:
        raise last_err

    out = np.empty((B, N, INNER), np.float32)
    for c in range(NCORES):
        b, hh = c // 2, c % 2
        out[b, :, hh * FQ:(hh + 1) * FQ] = res.results[c]["o"]
    return out
